# revision 26
# baseline (speedup 1.0000x reference)
"""Trainium2 Bass kernel for nn_Attention_11991548690893.

Reference semantics (faithfully-reproduced bug): q = k = v = the key
projection, so only the middle third of W_attn is used and the attention
matrix S = kh @ kh^T is SYMMETRIC.  Per-core plan (Megatron head-shard,
core c owns heads 2c, 2c+1 = 128 head-dims):

  - TRN2 power management caps the PE at ~50% issue rate while the ACT
    engine is near-saturated, releasing within ~1us.  exp() is therefore
    the commodity to minimize: S is EXPONENTIATED ONLY ON ITS UPPER
    TRIANGLE (58.6% of elements); the mirrored lower blocks are produced
    by PE transposes of the exp'd blocks (same PE cost as computing the
    logits, but zero ACT cost), batched through PSUM and placed with one
    strided DVE copy per row-block.  This keeps ACT duty under the
    throttle trigger so the PE streams at full 2.4 GHz.
  - All matmul operands 16-bit: fp16 k/weights, bf16 exp(S) (fp32 range,
    no-max-subtraction softmax cannot overflow).
  - Softmax denominators ride as a free ones-column in the out^T
    stationary (row 64 of the per-head PSUM accumulator).
  - PE-only phases (k-projection, output projection) are scheduled into
    ACT-quiet windows where matmuls run at full speed.
  - Projection partials stream out as fp16; host sums 8 partials + b_proj.
"""

import numpy as np

import concourse.bass as bass
import concourse.mybir as mybir
import concourse.tile as tile
from concourse import bacc
from concourse.bass_utils import run_bass_kernel_spmd

F32 = mybir.dt.float32
F16 = mybir.dt.float16
BF16 = mybir.dt.bfloat16

B = 2
L = 2048
D = 1024
H = 16
DH = 64
NCORES = 8
DHC = 128            # head-dims per core (2 heads x 64)
L2 = B * L           # 4096
P = 128
NBLK = L // P        # 16 l-blocks per batch
SCALE = 1.0 / np.sqrt(DH)   # 0.125


def _build_kernel(ctx, tc, xT, wk, bk, wp, ident_dram, out):
    nc = tc.nc

    singles = ctx.enter_context(tc.tile_pool(name="singles", bufs=1))
    xpool = ctx.enter_context(tc.tile_pool(name="xpool", bufs=2))
    rpool = ctx.enter_context(tc.tile_pool(name="rpool", bufs=2))
    bpool = ctx.enter_context(tc.tile_pool(name="bpool", bufs=2))
    ospool = ctx.enter_context(tc.tile_pool(name="ospool", bufs=2))
    opool = ctx.enter_context(tc.tile_pool(name="opool", bufs=3))
    ps_main = ctx.enter_context(tc.tile_pool(name="ps_main", bufs=2, space="PSUM"))
    ps_out = ctx.enter_context(tc.tile_pool(name="ps_out", bufs=1, space="PSUM"))
    dpool = ctx.enter_context(tc.tile_pool(name="dpool", bufs=4, space="DRAM"))

    ident32 = singles.tile([P, P], F32)
    nc.sync.dma_start(ident32, ident_dram)
    ident = singles.tile([P, P], F16)
    nc.vector.tensor_copy(ident, ident32)
    identBF = singles.tile([P, P], BF16)
    nc.vector.tensor_copy(identBF, ident32)

    wk_sb = singles.tile([P, 8, DHC], F16)    # W_k slice, D-major tiles
    nc.sync.dma_start(wk_sb, wk.rearrange("(o p) m -> p o m", p=P))
    bk_sb = singles.tile([P, 1], F32)
    nc.sync.dma_start(bk_sb, bk)
    wp_sb = singles.tile([P, D], F16)         # W_proj rows (128 dh of this core)
    nc.sync.dma_start(wp_sb, wp)

    kt = singles.tile([P, 8, 512], F16)       # kT = (x @ Wk + bk)^T, [dh, l]
    # k natural blocks + ones column per (lblk, head): [kh(64) | 1]
    knat = singles.tile([P, 32, 2, DH + 1], BF16)
    nc.vector.memset(knat[:, :, :, DH:DH + 1], 1.0)
    # exp(S) slabs for one (batch, head): slab i = S~[q-block i, all k]
    Sfull = singles.tile([P, NBLK, L], BF16)

    xTr = xT.rearrange("(o p) l -> p o l", p=P)   # [128, 8, 4096]

    def kproj_mm(lc):
        """k-projection matmuls + bias for kT chunk lc (512 l-cols)."""
        xc = xpool.tile([P, 8, 512], F16, tag="xc")
        nc.sync.dma_start(xc, xTr[:, :, lc * 512:(lc + 1) * 512])
        ps = ps_main.tile([P, 512], F32, tag="mm")
        for dc in range(8):
            nc.tensor.matmul(
                ps, wk_sb[:, dc], xc[:, dc], start=(dc == 0), stop=(dc == 7)
            )
        nc.vector.tensor_scalar_add(kt[:, lc], ps, bk_sb)

    def kproj_tr(lc):
        """knat blocks (kT transposes) for chunk lc."""
        for j in range(4):
            blk = lc * 4 + j
            tps = ps_main.tile([P, P], F16, tag="mm")
            nc.tensor.transpose(tps, kt[:, lc, j * P:(j + 1) * P], ident)
            nc.vector.tensor_copy(knat[:, blk, 0, 0:DH], tps[:, 0:DH])
            nc.vector.tensor_copy(knat[:, blk, 1, 0:DH], tps[:, DH:2 * DH])

    def kproj_range(lcs):
        prev = None
        for lc in lcs:
            kproj_mm(lc)
            if prev is not None:
                kproj_tr(prev)
            prev = lc
        kproj_tr(prev)

    def attention(b_, h2, oT, mid_work=None):
        """Triangle exp(S) + mirrors + out^T accumulation for (batch, head)."""

        def kt_cols(a, w):
            """kt slice for this (batch, head): global l-cols [a, a+w)."""
            return kt[:, b_ * 4 + a // 512][h2 * DH:(h2 + 1) * DH,
                                            a % 512:a % 512 + w]

        def att_block(i):
            """S logits + exp for blocks (i, j>=i): cols [i*128, 2048)."""
            c0 = i * P
            lhsT = kt_cols(c0, P)
            for ga, gb in ((c0, 1024), (max(c0, 1024), 2048)):
                if ga >= gb:
                    continue
                # anchor the aps to the 1024-aligned group start so every
                # matmul piece stays inside a PSUM bank
                g0 = (ga // 1024) * 1024
                aps = ps_main.tile([P, 1024], F32, tag="mm")
                p = ga
                while p < gb:
                    pw = min(512 - p % 512, gb - p)
                    nc.tensor.matmul(
                        aps[:, p - g0:p - g0 + pw],
                        lhsT,
                        kt_cols(p, pw),
                        start=True,
                        stop=True,
                    )
                    p += pw
                nc.scalar.activation(
                    Sfull[:, i, ga:gb],
                    aps[:, ga - g0:gb - g0],
                    mybir.ActivationFunctionType.Exp,
                    scale=SCALE,
                )

        def mirror_block(i):
            """Transpose blocks (i, j>i) into slabs j at col-block i."""
            n = NBLK - 1 - i
            if n == 0:
                return
            trT = ps_main.tile([P, 15 * P], BF16, tag="mm")
            for k in range(n):
                j = i + 1 + k
                nc.tensor.transpose(
                    trT[:, k * P:(k + 1) * P],
                    Sfull[:, i, j * P:(j + 1) * P],
                    identBF,
                )
            nc.vector.tensor_copy(
                Sfull[:, i + 1:NBLK, i * P:(i + 1) * P],
                trT[:, 0:n * P].rearrange("p (a b) -> p a b", b=P),
            )

        def outT_block(i):
            lhsT = knat[:, b_ * NBLK + i, h2]          # [128 l, 64+1]
            for qc in range(4):
                nc.tensor.matmul(
                    oT[0:DH + 1, qc * 512:(qc + 1) * 512],
                    lhsT,
                    Sfull[:, i, qc * 512:(qc + 1) * 512],
                    start=(i == 0),
                    stop=(i == NBLK - 1),
                    skip_group_check=True,
                )

        # pipeline: S/exp(i+1) emitted before mirrors(i) and outT(i); outT(i)
        # needs exp(i) plus mirror copies from blocks < i (done iterations ago).
        # depth-2 software pipeline: outT(i) runs two iterations behind the
        # mirror copy feeding it, so the PE never waits on the DVE drain.
        att_block(0)
        att_block(1)
        mirror_block(0)
        for i in range(2, NBLK):
            att_block(i)
            mirror_block(i - 1)
            outT_block(i - 2)
            if i == 8 and mid_work is not None:
                mid_work()   # PE-only filler dilutes ACT/DVE duty
        outT_block(NBLK - 2)
        outT_block(NBLK - 1)

    def normalize_head(oT, h2, osb2):
        """osb2 rows for head h2 = oT[0:64] / denom (denom = oT row 64).

        Chunked in 512-col pieces so the chain (recip -> DRAM roundtrip
        broadcast -> mul [-> shift]) pipelines and proj can start early."""
        recip = rpool.tile([1, L], F32, tag="rc")
        rdram = dpool.tile([1, L], F32)
        bcast = bpool.tile([DH, L], F32, tag=f"bc{h2}")
        osh = None
        if h2 == 1:
            osh = ospool.tile([DH, L], F16, tag="osh")
        for c in range(4):
            cs = slice(c * 512, (c + 1) * 512)
            nc.vector.reciprocal(recip[:, cs], oT[DH:DH + 1, cs])
            nc.sync.dma_start(rdram[:, cs], recip[:, cs])
            nc.sync.dma_start(
                bcast[:, cs],
                bass.AP(tensor=rdram.tensor, offset=rdram.offset + c * 512,
                        ap=[[0, DH], [1, 512]]),
            )
        for c in range(4):
            cs = slice(c * 512, (c + 1) * 512)
            if h2 == 0:
                nc.vector.tensor_mul(osb2[0:DH, cs], oT[0:DH, cs], bcast[:, cs])
            else:
                nc.vector.tensor_mul(osh[:, cs], oT[0:DH, cs], bcast[:, cs])
                nc.sync.dma_start(osb2[DH:2 * DH, cs], osh[:, cs])

    def proj(b_, osb2, tail):
        for qt in range(NBLK):
            pps = ps_main.tile([P, D], F32, tag="mm")
            for n2 in range(2):
                nc.tensor.matmul(
                    pps[:, n2 * 512:(n2 + 1) * 512],
                    osb2[:, qt * P:(qt + 1) * P],
                    wp_sb[:, n2 * 512:(n2 + 1) * 512],
                    start=True,
                    stop=True,
                )
            po = opool.tile([P, D], F16, tag="po")
            if qt % 2 == 1:
                nc.scalar.copy(po, pps)      # split copies: DVE + ACT
            else:
                nc.vector.tensor_copy(po, pps)
            nc.sync.dma_start(out[b_ * L + qt * P: b_ * L + (qt + 1) * P, :], po)

    # ---- schedule: PE-only phases inside ACT-quiet windows ----
    kproj_range(range(4))
    osb2_0 = ospool.tile([P, L], F16, tag="osb2")
    oT = ps_out.tile([P, L], F32, tag="ot")
    attention(0, 0, oT, mid_work=lambda: kproj_range(range(4, 8)))
    normalize_head(oT, 0, osb2_0)            # runs during att(0,1)
    oT = ps_out.tile([P, L], F32, tag="ot")
    attention(0, 1, oT)
    normalize_head(oT, 1, osb2_0)            # runs during att(1,0)
    osb2_1 = ospool.tile([P, L], F16, tag="osb2")
    oT = ps_out.tile([P, L], F32, tag="ot")
    attention(1, 0, oT)
    normalize_head(oT, 0, osb2_1)            # runs during proj(0)/att(1,1)
    proj(0, osb2_0, tail=False)              # ACT-quiet window: full-speed PE
    oT = ps_out.tile([P, L], F32, tag="ot")
    attention(1, 1, oT)
    normalize_head(oT, 1, osb2_1)
    proj(1, osb2_1, tail=True)


_NC_CACHE = None


def _get_nc():
    global _NC_CACHE
    if _NC_CACHE is None:
        nc = bacc.Bacc("TRN2", target_bir_lowering=False)
        xT = nc.dram_tensor("xt", [D, L2], F16, kind="ExternalInput").ap()
        wk = nc.dram_tensor("wk", [D, DHC], F16, kind="ExternalInput").ap()
        bk = nc.dram_tensor("bk", [DHC, 1], F32, kind="ExternalInput").ap()
        wp = nc.dram_tensor("wp", [DHC, D], F16, kind="ExternalInput").ap()
        ident = nc.dram_tensor("ident", [P, P], F32, kind="ExternalInput").ap()
        out = nc.dram_tensor("out", [L2, D], F16, kind="ExternalOutput").ap()
        from contextlib import ExitStack
        with tile.TileContext(nc) as tc, ExitStack() as ctx:
            _build_kernel(ctx, tc, xT, wk, bk, wp, ident, out)
        nc.compile()
        _NC_CACHE = nc
    return _NC_CACHE


def _run(inputs, trace=False):
    x = np.asarray(inputs["x"], dtype=np.float32)
    W_attn = np.asarray(inputs["W_attn"], dtype=np.float32)
    b_attn = np.asarray(inputs["b_attn"], dtype=np.float32)
    W_proj = np.asarray(inputs["W_proj"], dtype=np.float32)
    b_proj = np.asarray(inputs["b_proj"], dtype=np.float32)

    xT = np.ascontiguousarray(x.reshape(L2, D).T).astype(np.float16)
    Wk = W_attn[:, D:2 * D]                                  # [1024, 1024]
    bk = b_attn[D:2 * D]                                     # [1024]

    in_maps = []
    for c in range(NCORES):
        sl = slice(c * DHC, (c + 1) * DHC)
        in_maps.append({
            "xt": xT,
            "wk": np.ascontiguousarray(Wk[:, sl]).astype(np.float16),
            "bk": np.ascontiguousarray(bk[sl]).reshape(DHC, 1),
            "wp": np.ascontiguousarray(W_proj[sl, :]).astype(np.float16),
            "ident": np.eye(P, dtype=np.float32),
        })

    nc = _get_nc()
    res = run_bass_kernel_spmd(nc, in_maps, core_ids=list(range(NCORES)),
                               trace=trace)
    acc = res.results[0]["out"].astype(np.float64)
    for r in res.results[1:]:
        acc += r["out"]
    acc += b_proj
    return acc.astype(np.float32).reshape(B, L, D), res


def kernel(**inputs):
    out, _ = _run(inputs, trace=False)
    return out


def kernel_traced(**inputs):
    return _run(inputs, trace=True)


# revision 29
# speedup vs baseline: 1.0204x; 1.0204x over previous
"""Trainium2 Bass kernel for nn_Attention_11991548690893.

Reference semantics (faithfully-reproduced bug): q = k = v = the key
projection, so only the middle third of W_attn is used and the attention
matrix S = kh @ kh^T is SYMMETRIC.  Per-core plan (Megatron head-shard,
core c owns heads 2c, 2c+1 = 128 head-dims):

  - TRN2 power management caps the PE at ~50% issue rate while the ACT
    engine is near-saturated, releasing within ~1us.  exp() is therefore
    the commodity to minimize: S is EXPONENTIATED ONLY ON ITS UPPER
    TRIANGLE (58.6% of elements); the mirrored lower blocks are produced
    by PE transposes of the exp'd blocks (same PE cost as computing the
    logits, but zero ACT cost), batched through PSUM and placed with one
    strided DVE copy per row-block.  This keeps ACT duty under the
    throttle trigger so the PE streams at full 2.4 GHz.
  - All matmul operands 16-bit: fp16 k/weights, bf16 exp(S) (fp32 range,
    no-max-subtraction softmax cannot overflow).
  - Softmax denominators ride as a free ones-column in the out^T
    stationary (row 64 of the per-head PSUM accumulator).
  - PE-only phases (k-projection, output projection) are scheduled into
    ACT-quiet windows where matmuls run at full speed.
  - Projection partials stream out as fp16; host sums 8 partials + b_proj.
"""

import numpy as np

import concourse.bass as bass
import concourse.mybir as mybir
import concourse.tile as tile
from concourse import bacc
from concourse.bass_utils import run_bass_kernel_spmd

F32 = mybir.dt.float32
F16 = mybir.dt.float16
BF16 = mybir.dt.bfloat16

B = 2
L = 2048
D = 1024
H = 16
DH = 64
NCORES = 8
DHC = 128            # head-dims per core (2 heads x 64)
L2 = B * L           # 4096
P = 128
NBLK = L // P        # 16 l-blocks per batch
SCALE = 1.0 / np.sqrt(DH)   # 0.125


def _build_kernel(ctx, tc, xT, wk, bk, wp, ident_dram, out):
    nc = tc.nc

    singles = ctx.enter_context(tc.tile_pool(name="singles", bufs=1))
    xpool = ctx.enter_context(tc.tile_pool(name="xpool", bufs=2))
    rpool = ctx.enter_context(tc.tile_pool(name="rpool", bufs=2))
    bpool = ctx.enter_context(tc.tile_pool(name="bpool", bufs=2))
    ospool = ctx.enter_context(tc.tile_pool(name="ospool", bufs=2))
    otpool = ctx.enter_context(tc.tile_pool(name="otpool", bufs=2))
    opool = ctx.enter_context(tc.tile_pool(name="opool", bufs=3))
    ps_main = ctx.enter_context(tc.tile_pool(name="ps_main", bufs=2, space="PSUM"))
    ps_out = ctx.enter_context(tc.tile_pool(name="ps_out", bufs=1, space="PSUM"))
    dpool = ctx.enter_context(tc.tile_pool(name="dpool", bufs=4, space="DRAM"))

    ident32 = singles.tile([P, P], F32)
    nc.sync.dma_start(ident32, ident_dram)
    ident = singles.tile([P, P], F16)
    nc.vector.tensor_copy(ident, ident32)
    identBF = singles.tile([P, P], BF16)
    nc.vector.tensor_copy(identBF, ident32)

    wk_sb = singles.tile([P, 8, DHC], F16)    # W_k slice, D-major tiles
    nc.sync.dma_start(wk_sb, wk.rearrange("(o p) m -> p o m", p=P))
    bk_sb = singles.tile([P, 1], F32)
    nc.sync.dma_start(bk_sb, bk)
    wp_sb = singles.tile([P, D], F16)         # W_proj rows (128 dh of this core)
    nc.sync.dma_start(wp_sb, wp)

    kt = singles.tile([P, 8, 512], F16)       # kT = (x @ Wk + bk)^T, [dh, l]
    # k natural blocks + ones column per (lblk, head): [kh(64) | 1]
    knat = singles.tile([P, 32, 2, DH + 1], BF16)
    nc.vector.memset(knat[:, :, :, DH:DH + 1], 1.0)
    # exp(S) slabs for one (batch, head): slab i = S~[q-block i, all k]
    Sfull = singles.tile([P, NBLK, L], BF16)

    xTr = xT.rearrange("(o p) l -> p o l", p=P)   # [128, 8, 4096]

    def kproj_mm(lc):
        """k-projection matmuls + bias for kT chunk lc (512 l-cols)."""
        xc = xpool.tile([P, 8, 512], F16, tag="xc")
        nc.sync.dma_start(xc, xTr[:, :, lc * 512:(lc + 1) * 512])
        ps = ps_main.tile([P, 512], F32, tag="mm")
        for dc in range(8):
            nc.tensor.matmul(
                ps, wk_sb[:, dc], xc[:, dc], start=(dc == 0), stop=(dc == 7)
            )
        nc.vector.tensor_scalar_add(kt[:, lc], ps, bk_sb)

    def kproj_tr(lc):
        """knat blocks (kT transposes) for chunk lc."""
        for j in range(4):
            blk = lc * 4 + j
            tps = ps_main.tile([P, P], F16, tag="mm")
            nc.tensor.transpose(tps, kt[:, lc, j * P:(j + 1) * P], ident)
            nc.vector.tensor_copy(knat[:, blk, 0, 0:DH], tps[:, 0:DH])
            nc.vector.tensor_copy(knat[:, blk, 1, 0:DH], tps[:, DH:2 * DH])

    def kproj_range(lcs):
        prev = None
        for lc in lcs:
            kproj_mm(lc)
            if prev is not None:
                kproj_tr(prev)
            prev = lc
        kproj_tr(prev)

    def attention(b_, h2, oT, mid_work=None):
        """Triangle exp(S) + mirrors + out^T accumulation for (batch, head)."""

        def kt_cols(a, w):
            """kt slice for this (batch, head): global l-cols [a, a+w)."""
            return kt[:, b_ * 4 + a // 512][h2 * DH:(h2 + 1) * DH,
                                            a % 512:a % 512 + w]

        def att_block(i):
            """S logits + exp for blocks (i, j>=i): cols [i*128, 2048)."""
            c0 = i * P
            lhsT = kt_cols(c0, P)
            for ga, gb in ((c0, 1024), (max(c0, 1024), 2048)):
                if ga >= gb:
                    continue
                # anchor the aps to the 1024-aligned group start so every
                # matmul piece stays inside a PSUM bank
                g0 = (ga // 1024) * 1024
                aps = ps_main.tile([P, 1024], F32, tag="mm")
                p = ga
                while p < gb:
                    pw = min(512 - p % 512, gb - p)
                    nc.tensor.matmul(
                        aps[:, p - g0:p - g0 + pw],
                        lhsT,
                        kt_cols(p, pw),
                        start=True,
                        stop=True,
                    )
                    p += pw
                nc.scalar.activation(
                    Sfull[:, i, ga:gb],
                    aps[:, ga - g0:gb - g0],
                    mybir.ActivationFunctionType.Exp,
                    scale=SCALE,
                )

        def mirror_block(i):
            """Transpose blocks (i, j>i) into slabs j at col-block i."""
            n = NBLK - 1 - i
            if n == 0:
                return
            trT = ps_main.tile([P, 15 * P], BF16, tag="mm")
            for k in range(n):
                j = i + 1 + k
                nc.tensor.transpose(
                    trT[:, k * P:(k + 1) * P],
                    Sfull[:, i, j * P:(j + 1) * P],
                    identBF,
                )
            nc.vector.tensor_copy(
                Sfull[:, i + 1:NBLK, i * P:(i + 1) * P],
                trT[:, 0:n * P].rearrange("p (a b) -> p a b", b=P),
            )

        def outT_block(i):
            lhsT = knat[:, b_ * NBLK + i, h2]          # [128 l, 64+1]
            for qc in range(4):
                nc.tensor.matmul(
                    oT[0:DH + 1, qc * 512:(qc + 1) * 512],
                    lhsT,
                    Sfull[:, i, qc * 512:(qc + 1) * 512],
                    start=(i == 0),
                    stop=(i == NBLK - 1),
                    skip_group_check=True,
                )

        # pipeline: S/exp(i+1) emitted before mirrors(i) and outT(i); outT(i)
        # needs exp(i) plus mirror copies from blocks < i (done iterations ago).
        att_block(0)
        for i in range(1, NBLK):
            att_block(i)
            mirror_block(i - 1)
            outT_block(i - 1)
            if i == 8 and mid_work is not None:
                mid_work()   # PE-only filler dilutes ACT/DVE duty
        mirror_block(NBLK - 1)
        outT_block(NBLK - 1)

    def normalize_head(oT, h2, osb2):
        """osb2 rows for head h2 = oT[0:64] / denom (denom = oT row 64).

        oT is spilled to SBUF with ONE copy so the PSUM accumulator frees
        immediately for the next head; the rest of the chain (recip ->
        DRAM roundtrip broadcast -> mul on the idle GpSimd engine) runs
        entirely off SBUF, off the PE/DVE critical path."""
        tosb = otpool.tile([DH + 1, L], F32, tag="oTsb")
        nc.vector.tensor_copy(tosb, oT[0:DH + 1, :])
        recip = rpool.tile([1, L], F32, tag="rc")
        rdram = dpool.tile([1, L], F32)
        bcast = bpool.tile([DH, L], F32, tag=f"bc{h2}")
        osh = None
        if h2 == 1:
            osh = ospool.tile([DH, L], F16, tag="osh")
        for c in range(4):
            cs = slice(c * 512, (c + 1) * 512)
            nc.vector.reciprocal(recip[:, cs], tosb[DH:DH + 1, cs])
            nc.sync.dma_start(rdram[:, cs], recip[:, cs])
            nc.sync.dma_start(
                bcast[:, cs],
                bass.AP(tensor=rdram.tensor, offset=rdram.offset + c * 512,
                        ap=[[0, DH], [1, 512]]),
            )
        for c in range(4):
            cs = slice(c * 512, (c + 1) * 512)
            if h2 == 0:
                nc.gpsimd.tensor_mul(osb2[0:DH, cs], tosb[0:DH, cs], bcast[:, cs])
            else:
                nc.gpsimd.tensor_mul(osh[:, cs], tosb[0:DH, cs], bcast[:, cs])
                nc.sync.dma_start(osb2[DH:2 * DH, cs], osh[:, cs])

    def proj(b_, osb2, tail):
        for qt in range(NBLK):
            pps = ps_main.tile([P, D], F32, tag="mm")
            for n2 in range(2):
                nc.tensor.matmul(
                    pps[:, n2 * 512:(n2 + 1) * 512],
                    osb2[:, qt * P:(qt + 1) * P],
                    wp_sb[:, n2 * 512:(n2 + 1) * 512],
                    start=True,
                    stop=True,
                )
            po = opool.tile([P, D], F16, tag="po")
            if qt % 2 == 1:
                nc.scalar.copy(po, pps)      # split copies: DVE + ACT
            else:
                nc.vector.tensor_copy(po, pps)
            nc.sync.dma_start(out[b_ * L + qt * P: b_ * L + (qt + 1) * P, :], po)

    # ---- schedule: PE-only phases inside ACT-quiet windows ----
    kproj_range(range(4))
    osb2_0 = ospool.tile([P, L], F16, tag="osb2")
    oT = ps_out.tile([P, L], F32, tag="ot")
    attention(0, 0, oT, mid_work=lambda: kproj_range(range(4, 8)))
    normalize_head(oT, 0, osb2_0)            # runs during att(0,1)
    oT = ps_out.tile([P, L], F32, tag="ot")
    attention(0, 1, oT)
    normalize_head(oT, 1, osb2_0)            # runs during att(1,0)
    osb2_1 = ospool.tile([P, L], F16, tag="osb2")
    oT = ps_out.tile([P, L], F32, tag="ot")
    attention(1, 0, oT)
    normalize_head(oT, 0, osb2_1)            # runs during proj(0)/att(1,1)
    proj(0, osb2_0, tail=False)              # ACT-quiet window: full-speed PE
    oT = ps_out.tile([P, L], F32, tag="ot")
    attention(1, 1, oT)
    normalize_head(oT, 1, osb2_1)
    proj(1, osb2_1, tail=True)


_NC_CACHE = None


def _get_nc():
    global _NC_CACHE
    if _NC_CACHE is None:
        nc = bacc.Bacc("TRN2", target_bir_lowering=False)
        xT = nc.dram_tensor("xt", [D, L2], F16, kind="ExternalInput").ap()
        wk = nc.dram_tensor("wk", [D, DHC], F16, kind="ExternalInput").ap()
        bk = nc.dram_tensor("bk", [DHC, 1], F32, kind="ExternalInput").ap()
        wp = nc.dram_tensor("wp", [DHC, D], F16, kind="ExternalInput").ap()
        ident = nc.dram_tensor("ident", [P, P], F32, kind="ExternalInput").ap()
        out = nc.dram_tensor("out", [L2, D], F16, kind="ExternalOutput").ap()
        from contextlib import ExitStack
        with tile.TileContext(nc) as tc, ExitStack() as ctx:
            _build_kernel(ctx, tc, xT, wk, bk, wp, ident, out)
        nc.compile()
        _NC_CACHE = nc
    return _NC_CACHE


def _run(inputs, trace=False):
    x = np.asarray(inputs["x"], dtype=np.float32)
    W_attn = np.asarray(inputs["W_attn"], dtype=np.float32)
    b_attn = np.asarray(inputs["b_attn"], dtype=np.float32)
    W_proj = np.asarray(inputs["W_proj"], dtype=np.float32)
    b_proj = np.asarray(inputs["b_proj"], dtype=np.float32)

    xT = np.ascontiguousarray(x.reshape(L2, D).T).astype(np.float16)
    Wk = W_attn[:, D:2 * D]                                  # [1024, 1024]
    bk = b_attn[D:2 * D]                                     # [1024]

    in_maps = []
    for c in range(NCORES):
        sl = slice(c * DHC, (c + 1) * DHC)
        in_maps.append({
            "xt": xT,
            "wk": np.ascontiguousarray(Wk[:, sl]).astype(np.float16),
            "bk": np.ascontiguousarray(bk[sl]).reshape(DHC, 1),
            "wp": np.ascontiguousarray(W_proj[sl, :]).astype(np.float16),
            "ident": np.eye(P, dtype=np.float32),
        })

    nc = _get_nc()
    res = run_bass_kernel_spmd(nc, in_maps, core_ids=list(range(NCORES)),
                               trace=trace)
    acc = res.results[0]["out"].astype(np.float64)
    for r in res.results[1:]:
        acc += r["out"]
    acc += b_proj
    return acc.astype(np.float32).reshape(B, L, D), res


def kernel(**inputs):
    out, _ = _run(inputs, trace=False)
    return out


def kernel_traced(**inputs):
    return _run(inputs, trace=True)
